# revision 12
# baseline (speedup 1.0000x reference)
"""Trainium2 Bass kernel for ClassCentersEMA (vq_codebook).

Reference semantics (B=16384, D=1024, C=512):
    feats_n   = feats / max(||feats||_row, eps)
    counts    = targets.sum(0)                       # [C]
    class_sums= targets^T @ feats_n                  # [C, D]
    mask      = counts > 0
    means     = class_sums / max(counts, 1)          # rows 0 where !mask
    new       = !initialized & mask
    base      = where(new, means, centers)
    blended   = 0.9*base + 0.1*means
    upd       = where(mask, blended, centers)
    out       = where(mask, upd / max(||upd||, eps), upd)

Distribution: data-parallel over B across 8 cores. Each core computes
local [C] count and [C, D] class-sum partials; ReduceScatters hand core
r the 64 classes [64r, 64r+64); the epilogue runs on that shard and the
host concatenates the 8 [64, 1024] outputs.

Schedule: matmuls run in two D-half passes. Pass 0 streams feats/targets
from HBM (normalizing rows on the fly, tiles stay resident in SBUF) and
computes class_sums[:, 0:512]; its ReduceScatter overlaps pass 1, which
computes class_sums[:, 512:1024] from the resident tiles. Only the
second (1 MiB) ReduceScatter plus a short epilogue sit on the critical
tail. The counts ReduceScatter (2 KiB) is issued between the two big
ones and also overlaps pass 1.

The epilogue is algebraically folded to
    upd = a_c * centers + b_c * class_sums
with per-class scalars  (a, b):
    !mask:            (1.0, 0)
    mask & inited:    (0.9, 0.1/counts)
    mask & !inited:   (0.0, 1.0/counts)
followed by a masked renormalize. All count-derived scalars are
computed as soon as the counts ReduceScatter lands, off the tail.
"""

import numpy as np

import concourse.bass as bass
import concourse.mybir as mybir
import concourse.tile as tile
from concourse import bacc
from concourse.bass_utils import run_bass_kernel_spmd

F32 = mybir.dt.float32
F32R = mybir.dt.float32r
AF = mybir.ActivationFunctionType
ALU = mybir.AluOpType

NCORES = 8
B, D, C = 16384, 1024, 512
BL = B // NCORES          # 2048 rows per core
KT = BL // 128            # 16 k-tiles of 128
CL = C // NCORES          # 64 classes per core after ReduceScatter
MOM = 0.9
EPS = 1e-12


def build_nc(niters=1):
    """niters>1 unrolls the whole kernel body N times in one NEFF —
    used only for timing (slope over N isolates device exec time)."""
    nc = bacc.Bacc("TRN2", target_bir_lowering=False, debug=False,
                   num_devices=NCORES)

    feats = nc.dram_tensor("feats", [BL, D], F32, kind="ExternalInput")
    targets = nc.dram_tensor("targets", [BL, C], F32, kind="ExternalInput")
    centers = nc.dram_tensor("centers", [CL, D], F32, kind="ExternalInput")
    inited = nc.dram_tensor("inited", [CL, 1], F32, kind="ExternalInput")
    out = nc.dram_tensor("out", [CL, D], F32, kind="ExternalOutput")

    rg = [list(range(NCORES))]

    with tile.TileContext(nc) as tc:
        with (
            tc.tile_pool(name="dram", bufs=1, space="DRAM") as dram,
            tc.tile_pool(name="ftp", bufs=3) as ftp,
            tc.tile_pool(name="res", bufs=1) as res,
            tc.tile_pool(name="sq", bufs=2) as sqp,
            tc.tile_pool(name="small", bufs=6) as small,
            tc.tile_pool(name="single", bufs=1) as single,
            tc.tile_pool(name="cs", bufs=4) as csp,
            tc.tile_pool(name="psum", bufs=8, space="PSUM") as ppool,
            tc.tile_pool(name="epi", bufs=1) as epi,
        ):
            io = dict(feats=feats, targets=targets, centers=centers,
                      inited=inited, out=out)
            bounces = dict(
                rs_in_a=dram.tile([C, 512], F32, name="rs_in_a"),
                rs_out_a=dram.tile([CL, 512], F32, name="rs_out_a"),
                rs_in_b=dram.tile([C, 512], F32, name="rs_in_b"),
                rs_out_b=dram.tile([CL, 512], F32, name="rs_out_b"),
                cnt_in=dram.tile([1, C], F32, name="cnt_in"),
                cnt_out=dram.tile([CL, 1], F32, name="cnt_out"),
            )

            ones_f = single.tile([128, 1], F32)
            nc.vector.memset(ones_f[:], 1.0)
            ones = single.tile([128, 1], F32R)
            nc.vector.tensor_copy(ones[:], ones_f[:])
            consts = dict(ones=ones,
                          acc_tg=single.tile([128, C], F32, name="acc_tg"),
                          acc_tgr=single.tile([128, C], F32R,
                                              name="acc_tgr"))

            pools = dict(ftp=ftp, res=res, sqp=sqp, small=small, csp=csp,
                         ppool=ppool, epi=epi)
            for _ in range(niters):
                _emit_iteration(nc, tc, io, bounces, consts, pools, rg)

    nc.compile()
    return nc


def _emit_iteration(nc, tc, io, bb, consts, pools, rg):
    feats, targets = io["feats"], io["targets"]
    centers, inited, out = io["centers"], io["inited"], io["out"]
    ones = consts["ones"]
    acc_tg, acc_tgr = consts["acc_tg"], consts["acc_tgr"]
    ftp, res, sqp = pools["ftp"], pools["res"], pools["sqp"]
    small, csp, ppool, epi = (pools["small"], pools["csp"], pools["ppool"],
                              pools["epi"])

    # epilogue inputs that depend on nothing — issue DMAs up front
    ctr = epi.tile([CL, D], F32, tag="ctr")
    nc.sync.dma_start(ctr[:], centers[:])
    ini = epi.tile([CL, 1], F32, tag="ini")
    nc.sync.dma_start(ini[:], inited[:])

    # PSUM accumulators: pass a = D[0:512], pass b = D[512:1024]
    ps_a = [ppool.tile([128, 512], F32, tag="acc", name=f"ps_a{c}")
            for c in range(4)]
    ps_b = [ppool.tile([128, 512], F32, tag="acc", name=f"ps_b{c}")
            for c in range(4)]

    # DMA 2 k-tiles per transfer (~1 MiB): partition p, slot a holds
    # DRAM row (2q+a)*128 + p
    feats_t = feats.rearrange("(a p) d -> p a d", p=128)
    targets_t = targets.rearrange("(a p) c -> p a c", p=128)

    # ---- pass a: stream + normalize (resident) + matmul D[0:512] ----
    ftrs, tgs = [], []
    for q in range(KT // 2):
        ft2 = ftp.tile([128, 2, D], F32, tag="ft")
        nc.sync.dma_start(ft2[:], feats_t[:, 2 * q:2 * q + 2, :])
        # targets are exactly 0/1 -> fp32r rounding is the identity,
        # so a bitcast byte-copy into an f32r tile is exact
        tg2 = res.tile([128, 2, C], F32R, tag="tg", bufs=KT // 2,
                       name=f"tg{q}")
        nc.sync.dma_start(tg2[:],
                          targets_t[:, 2 * q:2 * q + 2, :].bitcast(F32R))
        tgs.append(tg2)
        for a in range(2):
            k = 2 * q + a
            ft = ft2[:, a, :]
            tg = tg2[:, a, :]
            # row norms: ssq = sum(ft^2); rcp = 1/max(sqrt(ssq), eps)
            sq = sqp.tile([128, D], F32, tag="sq")
            ssq = small.tile([128, 1], F32, tag="ssq")
            nc.scalar.activation(sq[:], ft, AF.Square, accum_out=ssq[:])
            nrm = small.tile([128, 1], F32, tag="nrm")
            nc.scalar.activation(nrm[:], ssq[:], AF.Sqrt)
            nc.vector.tensor_scalar_max(nrm[:], nrm[:], EPS)
            rcp = small.tile([128, 1], F32, tag="rcp")
            nc.vector.reciprocal(rcp[:], nrm[:])
            # normalized feats, rounded to f32r by the DVE; stays resident
            ftr = res.tile([128, D], F32R, tag="ftr", bufs=KT,
                           name=f"ftr{k}")
            nc.vector.tensor_scalar_mul(ftr[:], ft, rcp[:])
            ftrs.append(ftr)

            # accumulate raw targets for counts
            if k == 0:
                nc.vector.tensor_copy(acc_tg[:], tg.bitcast(F32))
            else:
                nc.vector.tensor_add(acc_tg[:], acc_tg[:], tg.bitcast(F32))

            for c in range(4):
                nc.tensor.matmul(ps_a[c][:], tg[:, c * 128:(c + 1) * 128],
                                 ftr[:, 0:512],
                                 start=(k == 0), stop=(k == KT - 1))

    # drain pass-a PSUM and start its ReduceScatter (overlaps pass b)
    for c in range(4):
        cs_sb = csp.tile([128, 512], F32, tag="cs_sb")
        if c % 2 == 0:
            nc.vector.tensor_copy(cs_sb[:], ps_a[c][:])
        else:
            nc.scalar.copy(cs_sb[:], ps_a[c][:])
        nc.sync.dma_start(bb["rs_in_a"][c * 128:(c + 1) * 128, :], cs_sb[:])
    nc.gpsimd.collective_compute(
        "ReduceScatter", ALU.add, replica_groups=rg,
        ins=[bb["rs_in_a"][:].opt()], outs=[bb["rs_out_a"][:].opt()])

    # counts = ones^T @ acc_tg -> [1, C]; tiny ReduceScatter, overlaps pass b
    nc.scalar.copy(acc_tgr[:], acc_tg[:])
    cps = ppool.tile([1, C], F32, tag="acc")
    nc.tensor.matmul(cps[:], ones[:], acc_tgr[:], start=True, stop=True)
    cnt_sb = small.tile([1, C], F32, tag="cnt_sb")
    nc.vector.tensor_copy(cnt_sb[:], cps[:])
    nc.sync.dma_start(bb["cnt_in"][:], cnt_sb[:])
    nc.gpsimd.collective_compute(
        "ReduceScatter", ALU.add, replica_groups=rg,
        ins=[bb["cnt_in"][:].opt()], outs=[bb["cnt_out"][:].opt()])

    # ---- pass b: matmul D[512:1024] from resident tiles ----
    for k in range(KT):
        tg = tgs[k // 2][:, k % 2, :]
        ftr = ftrs[k]
        for c in range(4):
            nc.tensor.matmul(ps_b[c][:], tg[:, c * 128:(c + 1) * 128],
                             ftr[:, 512:1024],
                             start=(k == 0), stop=(k == KT - 1))

    for c in range(4):
        cs_sb = csp.tile([128, 512], F32, tag="cs_sb")
        if c % 2 == 0:
            nc.vector.tensor_copy(cs_sb[:], ps_b[c][:])
        else:
            nc.scalar.copy(cs_sb[:], ps_b[c][:])
        nc.sync.dma_start(bb["rs_in_b"][c * 128:(c + 1) * 128, :], cs_sb[:])
    nc.gpsimd.collective_compute(
        "ReduceScatter", ALU.add, replica_groups=rg,
        ins=[bb["rs_in_b"][:].opt()], outs=[bb["rs_out_b"][:].opt()])

    # ---- epilogue on this core's CL classes ----
    # count-derived per-class scalars: ready as soon as the tiny RS lands
    cnt = epi.tile([CL, 1], F32, tag="cnt")
    nc.sync.dma_start(cnt[:], bb["cnt_out"][:])
    mask = epi.tile([CL, 1], F32, tag="mask")
    nc.vector.tensor_scalar_min(mask[:], cnt[:], 1.0)
    inv = epi.tile([CL, 1], F32, tag="inv")
    nc.vector.tensor_scalar_max(inv[:], cnt[:], 1.0)
    nc.vector.reciprocal(inv[:], inv[:])
    # new01 = (1 - inited) * mask
    new01 = epi.tile([CL, 1], F32, tag="new01")
    nc.vector.tensor_scalar(new01[:], ini[:], -1.0, 1.0,
                            op0=ALU.mult, op1=ALU.add)
    nc.vector.tensor_mul(new01[:], new01[:], mask[:])
    # b = mask * (0.1 + 0.9*new01); a = 1 - b; bp = b / max(cnt,1)
    bco = epi.tile([CL, 1], F32, tag="bco")
    nc.vector.tensor_scalar(bco[:], new01[:], MOM, 1.0 - MOM,
                            op0=ALU.mult, op1=ALU.add)
    nc.vector.tensor_mul(bco[:], bco[:], mask[:])
    aco = epi.tile([CL, 1], F32, tag="aco")
    nc.vector.tensor_scalar(aco[:], bco[:], -1.0, 1.0,
                            op0=ALU.mult, op1=ALU.add)
    nc.vector.tensor_mul(bco[:], bco[:], inv[:])
    # base = a*centers, ready before the big RS lands
    upd = epi.tile([CL, D], F32, tag="upd")
    nc.vector.tensor_scalar_mul(upd[:], ctr[:], aco[:])

    cs = epi.tile([CL, D], F32, tag="cs")
    nc.sync.dma_start(cs[:, 0:512], bb["rs_out_a"][:])
    nc.sync.dma_start(cs[:, 512:1024], bb["rs_out_b"][:])

    # upd += bp*class_sums  (half a: off the tail; half b: the tail)
    nc.vector.scalar_tensor_tensor(upd[:, 0:512], cs[:, 0:512], bco[:],
                                   upd[:, 0:512], op0=ALU.mult, op1=ALU.add)
    nc.vector.scalar_tensor_tensor(upd[:, 512:1024], cs[:, 512:1024], bco[:],
                                   upd[:, 512:1024],
                                   op0=ALU.mult, op1=ALU.add)

    # masked renormalize
    usq = epi.tile([CL, D], F32, tag="usq")
    ussq = epi.tile([CL, 1], F32, tag="ussq")
    nc.scalar.activation(usq[:], upd[:], AF.Square, accum_out=ussq[:])
    unrm = epi.tile([CL, 1], F32, tag="unrm")
    nc.scalar.activation(unrm[:], ussq[:], AF.Sqrt)
    nc.vector.tensor_scalar_max(unrm[:], unrm[:], EPS)
    urcp = epi.tile([CL, 1], F32, tag="urcp")
    nc.vector.reciprocal(urcp[:], unrm[:])
    # rfin = mask*(urcp - 1) + 1
    nc.vector.scalar_tensor_tensor(urcp[:], urcp[:], 1.0, mask[:],
                                   op0=ALU.subtract, op1=ALU.mult)
    nc.vector.tensor_scalar_add(urcp[:], urcp[:], 1.0)

    ov = epi.tile([CL, D], F32, tag="ov")
    nc.vector.tensor_scalar_mul(ov[:], upd[:], urcp[:])
    nc.sync.dma_start(out[:], ov[:])


_NC_CACHE = None


def _get_nc():
    global _NC_CACHE
    if _NC_CACHE is None:
        _NC_CACHE = build_nc()
    return _NC_CACHE


def run_spmd(feats, targets, centers, initialized, **kw):
    feats = np.ascontiguousarray(np.asarray(feats, dtype=np.float32))
    targets = np.ascontiguousarray(np.asarray(targets, dtype=np.float32))
    centers = np.ascontiguousarray(np.asarray(centers, dtype=np.float32))
    init_f = np.asarray(initialized).astype(np.float32).reshape(C, 1)
    assert feats.shape == (B, D) and targets.shape == (B, C)
    assert centers.shape == (C, D)

    nc = _get_nc()
    in_maps = []
    for r in range(NCORES):
        in_maps.append({
            "feats": feats[r * BL:(r + 1) * BL],
            "targets": targets[r * BL:(r + 1) * BL],
            "centers": np.ascontiguousarray(centers[r * CL:(r + 1) * CL]),
            "inited": np.ascontiguousarray(init_f[r * CL:(r + 1) * CL]),
        })
    res = run_bass_kernel_spmd(nc, in_maps, core_ids=list(range(NCORES)), **kw)
    out = np.concatenate([res.results[r]["out"] for r in range(NCORES)], axis=0)
    return out.astype(np.float32), res


def kernel(feats, targets, centers, initialized):
    out, _ = run_spmd(feats, targets, centers, initialized)
    return out
